# revision 1
# baseline (speedup 1.0000x reference)
"""Trainium2 Bass kernel for nn_Luong_61684320305412 (bidirectional masked
softmax attention, B=8, L0=L1=2048, D=256).

Sharding: data-parallel over batch B across the 8 NeuronCores (one batch
element per core). Per core:

    S      = q0 @ q1^T * (1/256) + NEG * mask0[:,None]*mask1[None,:]
    E      = exp(S)            (no max-subtraction needed: |S_unmasked| << 80,
                                masked entries underflow to exactly 0)
    out0   = (E @ q1) * (1/16) / rowsum(E)[:, None]
    out1   = (E^T @ q0) * (1/16) / colsum(E)[None, :]^T

Implementation notes:
  - The mask outer product is folded into the score matmul as a rank-1
    augmented contraction: an extra K=1 matmul with lhsT = -2^17*mask_l,
    rhs = +2^17*mask_r, so exp sees -2^26 on masked entries -> exactly 0.
  - Row/col sums come from an appended ones-column in the rhs of the
    out-matmuls (psum column D holds the softmax denominator).
  - E is needed with both orientations on the partition axis; we compute
    S twice (S and S^T) from transposed copies of q0/q1 rather than
    transposing the 2048x2048 E.
  - All matmuls use float32r (full-rate fp32 path, 1 cycle/row for N>=256).
  - L1 (resp. L0) is processed in halves so only half of E (8 MB) is
    resident in SBUF at a time.
"""

import math
from contextlib import ExitStack

import numpy as np

import concourse.bass as bass
import concourse.tile as tile
from concourse import bacc, mybir
from concourse.bass_utils import run_bass_kernel_spmd
from concourse.masks import make_identity

P = 128
B = 8
L = 2048          # L0 == L1
D = 256
T = L // P        # 16 row tiles
DC = D // P       # 2 contraction chunks of 128
HALF = L // 2     # 1024
NCHUNK = 512      # psum bank width in fp32
AUGW = D + 2      # 258: q-tiles augmented with two ones columns (even N for fp32r)
MASKC = 131072.0  # 2^17; (-2^17 m0)*(2^17 m1)/256 = -2^26 -> exp underflows to 0
SCALE2 = 1.0 / 256.0   # applied to scores inside exp
SCALE1 = 1.0 / 16.0    # applied to the averaged values at the end

f32 = mybir.dt.float32
f32r = mybir.dt.float32r
i32 = mybir.dt.int32
MUL = mybir.AluOpType.mult
EXP = mybir.ActivationFunctionType.Exp


def _emit(tc: tile.TileContext, ctx: ExitStack, io: dict):
    nc = tc.nc
    q0, q1, m0, m1 = io["q0"], io["q1"], io["mask0"], io["mask1"]
    out0, out1 = io["out0"], io["out1"]

    consts = ctx.enter_context(tc.tile_pool(name="consts", bufs=1))
    qaug = ctx.enter_context(tc.tile_pool(name="qaug", bufs=1))
    qT = ctx.enter_context(tc.tile_pool(name="qT", bufs=1))
    e_pool = ctx.enter_context(tc.tile_pool(name="e", bufs=18))
    outp = ctx.enter_context(tc.tile_pool(name="outp", bufs=4))
    small = ctx.enter_context(tc.tile_pool(name="small", bufs=4))
    t_psum = ctx.enter_context(tc.tile_pool(name="t_psum", bufs=2, space="PSUM"))
    s_psum = ctx.enter_context(tc.tile_pool(name="s_psum", bufs=2, space="PSUM"))
    o_psum = ctx.enter_context(tc.tile_pool(name="o_psum", bufs=2, space="PSUM"))

    # ---- load q0/q1 into augmented layout [p, t, D+2] (ones columns at D, D+1;
    # width D+2=258 keeps the fp32r matmul moving-dim even) ----
    q0a = qaug.tile([P, T, AUGW], f32r)
    q1a = qaug.tile([P, T, AUGW], f32r)
    nc.sync.dma_start(
        out=q0a[:, :, 0:D], in_=q0.rearrange("(t p) d -> p t d", p=P).bitcast(f32r)
    )
    nc.sync.dma_start(
        out=q1a[:, :, 0:D], in_=q1.rearrange("(t p) d -> p t d", p=P).bitcast(f32r)
    )
    # memset can't write f32r; stage ones in f32 and round via tensor_copy
    ones_f = consts.tile([P, T, 2], f32)
    nc.vector.memset(ones_f, 1.0)
    nc.vector.tensor_copy(out=q0a[:, :, D:AUGW], in_=ones_f)
    nc.vector.tensor_copy(out=q1a[:, :, D:AUGW], in_=ones_f)

    # ---- masks: int32 [L] -> f32 rows scaled by -+2^17 ----
    # (separate [1, L] tiles: matmul operands must start at partition 0)
    m0i = consts.tile([1, L], i32)
    m1i = consts.tile([1, L], i32)
    nc.sync.dma_start(out=m0i, in_=m0.rearrange("(o l) -> o l", o=1))
    nc.sync.dma_start(out=m1i, in_=m1.rearrange("(o l) -> o l", o=1))
    m0f = consts.tile([1, L], f32r)
    m1f = consts.tile([1, L], f32r)
    nc.vector.tensor_copy(out=m0f, in_=m0i)  # int32 -> fp32 cast
    nc.vector.tensor_copy(out=m1f, in_=m1i)
    nc.vector.tensor_scalar_mul(out=m0f, in0=m0f, scalar1=-MASKC)
    nc.vector.tensor_scalar_mul(out=m1f, in0=m1f, scalar1=MASKC)
    mrows = (m0f, m1f)

    # ---- transpose q0/q1 (data part) to [d-part, l] layout via PE ----
    ident_f = consts.tile([P, P], f32)
    make_identity(nc, ident_f)
    ident = consts.tile([P, P], f32r)
    nc.vector.tensor_copy(out=ident, in_=ident_f)
    q0t = qT.tile([P, DC, L], f32r)
    q1t = qT.tile([P, DC, L], f32r)
    for src, dst in ((q0a, q0t), (q1a, q1t)):
        for t in range(T):
            for dc in range(DC):
                pt = t_psum.tile([P, P], f32r, tag="tp")
                nc.tensor.transpose(pt, src[:, t, dc * P : (dc + 1) * P], ident)
                nc.vector.tensor_copy(out=dst[:, dc, t * P : (t + 1) * P], in_=pt)

    # ---- main phases ----
    # orient 0: rows of E = l0 (feeds out1);  orient 1: rows of E^T = l1 (feeds out0)
    for orient in range(2):
        if orient == 0:
            lT, rT = q0t, q1t
            lm, rm = 0, 1
            raug = q0a
            odram = out1
        else:
            lT, rT = q1t, q0t
            lm, rm = 1, 0
            raug = q1a
            odram = out0
        for h in range(2):
            etiles = []
            for t in range(T):
                ps = s_psum.tile([P, HALF], f32, tag="sp")
                for c in range(HALF // NCHUNK):
                    off = h * HALF + c * NCHUNK
                    sl = ps[:, c * NCHUNK : (c + 1) * NCHUNK]
                    for dc in range(DC):
                        nc.tensor.matmul(
                            sl,
                            lhsT=lT[:, dc, t * P : (t + 1) * P],
                            rhs=rT[:, dc, off : off + NCHUNK],
                            start=(dc == 0),
                            stop=False,
                        )
                    nc.tensor.matmul(
                        sl,
                        lhsT=mrows[lm][:, t * P : (t + 1) * P],
                        rhs=mrows[rm][:, off : off + NCHUNK],
                        start=False,
                        stop=True,
                    )
                et = e_pool.tile([P, HALF], f32r, tag="E")
                nc.scalar.activation(out=et, in_=ps, func=EXP, scale=SCALE2)
                etiles.append(et)
            for mt in range(HALF // P):
                po = o_psum.tile([P, AUGW], f32, tag="op")
                for t in range(T):
                    nc.tensor.matmul(
                        po,
                        lhsT=etiles[t][:, mt * P : (mt + 1) * P],
                        rhs=raug[:, t, :],
                        start=(t == 0),
                        stop=(t == T - 1),
                    )
                rc = small.tile([P, 1], f32, tag="rc")
                nc.vector.reciprocal(rc, po[:, D : D + 1])
                ot = outp.tile([P, D], f32, tag="ot")
                nc.vector.tensor_scalar(
                    out=ot,
                    in0=po[:, 0:D],
                    scalar1=rc,
                    scalar2=SCALE1,
                    op0=MUL,
                    op1=MUL,
                )
                row = h * HALF + mt * P
                nc.sync.dma_start(out=odram[row : row + P, :], in_=ot)


_CACHED_NC = None


def _build():
    global _CACHED_NC
    if _CACHED_NC is not None:
        return _CACHED_NC
    nc = bacc.Bacc("TRN2", target_bir_lowering=False, debug=False)
    io = {
        "q0": nc.dram_tensor("q0", [L, D], f32, kind="ExternalInput").ap(),
        "q1": nc.dram_tensor("q1", [L, D], f32, kind="ExternalInput").ap(),
        "mask0": nc.dram_tensor("mask0", [L], i32, kind="ExternalInput").ap(),
        "mask1": nc.dram_tensor("mask1", [L], i32, kind="ExternalInput").ap(),
        "out0": nc.dram_tensor("out0", [L, D], f32, kind="ExternalOutput").ap(),
        "out1": nc.dram_tensor("out1", [L, D], f32, kind="ExternalOutput").ap(),
    }
    with tile.TileContext(nc) as tc:
        with ExitStack() as ctx:
            _emit(tc, ctx, io)
    nc.compile()
    _CACHED_NC = nc
    return nc


def run_on_cores(q0, q1, mask0, mask1, trace=False):
    """Run the SPMD kernel; returns (out0, out1, BassKernelResults)."""
    nc = _build()
    in_maps = [
        {
            "q0": np.ascontiguousarray(q0[b], dtype=np.float32),
            "q1": np.ascontiguousarray(q1[b], dtype=np.float32),
            "mask0": np.ascontiguousarray(mask0[b], dtype=np.int32),
            "mask1": np.ascontiguousarray(mask1[b], dtype=np.int32),
        }
        for b in range(B)
    ]
    br = run_bass_kernel_spmd(nc, in_maps, list(range(B)), trace=trace)
    out0 = np.stack([br.results[b]["out0"] for b in range(B)])
    out1 = np.stack([br.results[b]["out1"] for b in range(B)])
    return out0, out1, br


def kernel(q0, q1, len0=None, len1=None, mask0=None, mask1=None, **_):
    q0 = np.asarray(q0, dtype=np.float32)
    q1 = np.asarray(q1, dtype=np.float32)
    mask0 = np.asarray(mask0, dtype=np.int32)
    mask1 = np.asarray(mask1, dtype=np.int32)
    out0, out1, _br = run_on_cores(q0, q1, mask0, mask1, trace=False)
    return out0, out1



# revision 3
# speedup vs baseline: 1.0574x; 1.0574x over previous
"""Trainium2 Bass kernel for nn_Luong_61684320305412 (bidirectional masked
softmax attention, B=8, L0=L1=2048, D=256).

Sharding: data-parallel over batch B across the 8 NeuronCores (one batch
element per core). Per core:

    S      = q0 @ q1^T + (-240 m0) outer (240 m1)     [fp8 DoubleRow matmuls]
    E      = exp(S / 256)          (masked entries see -225 -> exp == 0;
                                    |S/256| <= ~0.35 so no max-subtraction)
    out0   = (E @ q1) * (1/16) / rowsum(E)
    out1   = (E^T @ q0) * (1/16) / colsum(E)

Implementation:
  - Scores are computed ONCE in fp8e4 (DoubleRow perf mode: K=256 per
    instruction, 2 columns/cycle). The mask outer product is a K=1
    DoubleRow matmul accumulated into the same PSUM chunk.
  - E is materialized in fp16 (full 2048x2048, 8 MB SBUF); E^T is obtained
    with PE transposes (fp16, 1 col/cycle) instead of recomputing S^T,
    which halves the scalar-engine exp work.
  - The out matmuls run in fp16 (1 col/cycle). Row/col sums come from two
    ones-columns appended to the rhs q tiles (psum column D holds the
    softmax denominator).
  - Normalization: reciprocal + one-op tensor_scalar on DVE, fp32 out.
"""

import math
from contextlib import ExitStack

import numpy as np

import concourse.bass as bass
import concourse.tile as tile
from concourse import bacc, mybir
from concourse.bass_utils import run_bass_kernel_spmd
from concourse.masks import make_identity

P = 128
B = 8
L = 2048          # L0 == L1
D = 256
T = L // P        # 16 row tiles
NCHUNK = 512      # psum bank width in fp32
NC_PER_T = L // NCHUNK   # 4 chunks per row tile
AUGW = D + 2      # 258: q tiles augmented with two ones columns
MASKC = 240.0     # (-240 m0)*(240 m1)/256 = -225 -> exp underflows to 0
SCALE2 = 1.0 / 256.0   # applied to scores inside exp
SCALE1 = 1.0 / 16.0    # applied to the averaged values at the end

f32 = mybir.dt.float32
f16 = mybir.dt.float16
f8 = mybir.dt.float8e4
i32 = mybir.dt.int32
EXP = mybir.ActivationFunctionType.Exp
DR = mybir.MatmulPerfMode.DoubleRow


def _emit(tc: tile.TileContext, ctx: ExitStack, io: dict):
    nc = tc.nc
    q0, q1, m0, m1 = io["q0"], io["q1"], io["mask0"], io["mask1"]
    out0, out1 = io["out0"], io["out1"]

    consts = ctx.enter_context(tc.tile_pool(name="consts", bufs=1))
    qaug = ctx.enter_context(tc.tile_pool(name="qaug", bufs=1))
    qT = ctx.enter_context(tc.tile_pool(name="qT", bufs=1))
    e_pool = ctx.enter_context(tc.tile_pool(name="e", bufs=1))
    outp = ctx.enter_context(tc.tile_pool(name="outp", bufs=4))
    small = ctx.enter_context(tc.tile_pool(name="small", bufs=4))
    stage = ctx.enter_context(tc.tile_pool(name="stage", bufs=1))
    t_psum = ctx.enter_context(tc.tile_pool(name="t_psum", bufs=2, space="PSUM"))
    s_psum = ctx.enter_context(tc.tile_pool(name="s_psum", bufs=3, space="PSUM"))
    o_psum = ctx.enter_context(tc.tile_pool(name="o_psum", bufs=2, space="PSUM"))

    # ---- identity for PE transposes (fp16) ----
    ident_f = consts.tile([P, P], f32)
    make_identity(nc, ident_f)
    ident = consts.tile([P, P], f16)
    nc.gpsimd.tensor_copy(out=ident, in_=ident_f)

    # ---- masks: int32 [L] -> fp8 rows valued -+240, packed [1, 2, L] with
    # the second k-slot zeroed (DoubleRow contracts both slots) ----
    m0i = consts.tile([1, L], i32)
    m1i = consts.tile([1, L], i32)
    nc.sync.dma_start(out=m0i, in_=m0.rearrange("(o l) -> o l", o=1))
    nc.sync.dma_start(out=m1i, in_=m1.rearrange("(o l) -> o l", o=1))
    m0r8 = consts.tile([1, 2, L], f8)
    m1r8 = consts.tile([1, 2, L], f8)
    mf = consts.tile([1, L], f32)
    nc.gpsimd.memset(m0r8, 0.0)
    nc.gpsimd.memset(m1r8, 0.0)
    nc.gpsimd.tensor_copy(out=mf, in_=m0i)
    nc.gpsimd.tensor_scalar_mul(out=m0r8[:, 0, :], in0=mf, scalar1=-MASKC)
    nc.gpsimd.tensor_copy(out=mf, in_=m1i)
    nc.gpsimd.tensor_scalar_mul(out=m1r8[:, 0, :], in0=mf, scalar1=MASKC)

    # ---- load q0/q1 (f32) and cast to fp16 augmented tiles [p, t, D+2] ----
    q0a = qaug.tile([P, T, AUGW], f16)
    q1a = qaug.tile([P, T, AUGW], f16)
    q0t8 = qT.tile([P, 2, L], f8)   # [d%128, d//128, l] fp8 packed (DoubleRow)
    q1t8 = qT.tile([P, 2, L], f8)
    nc.vector.memset(q0a[:, :, D:AUGW], 1.0)
    nc.vector.memset(q1a[:, :, D:AUGW], 1.0)
    for src_dram, dst_a, dst_t8 in ((q0, q0a, q0t8), (q1, q1a, q1t8)):
        qf = stage.tile([P, T, D], f32, tag="qstage")
        nc.sync.dma_start(out=qf, in_=src_dram.rearrange("(t p) d -> p t d", p=P))
        nc.scalar.copy(out=dst_a[:, :, 0:D], in_=qf)
        # transpose (fp16) -> evict-cast to fp8 packed layout
        for t in range(T):
            pt = t_psum.tile([P, 4, P], f16, tag="tp")
            for dc in range(2):
                nc.tensor.transpose(
                    pt[:, dc, :], dst_a[:, t, dc * P : (dc + 1) * P], ident
                )
            nc.vector.tensor_copy(
                out=dst_t8[:, :, t * P : (t + 1) * P], in_=pt[:, 0:2, :]
            )

    # ---- S-phase: E = exp((q0 q1^T + mask)/256), one orientation only;
    # E^T transposes for finished 4-row-tile groups are interleaved ----
    E = e_pool.tile([P, T, L], f16)       # [l0%128, l0//128, l1]
    Et = e_pool.tile([P, T, T, P], f16)   # [l1%128, l1//128, l0//128, l0%128]

    def emit_et_batch(g, t1_lo):
        # transpose E rows g*4..g*4+3 for t1 tiles t1_lo..t1_lo+3
        for t1 in range(t1_lo, t1_lo + 4):
            pt = t_psum.tile([P, 4, P], f16, tag="tp")
            for tq in range(4):
                nc.tensor.transpose(
                    pt[:, tq, :],
                    E[:, g * 4 + tq, t1 * P : (t1 + 1) * P],
                    ident,
                )
            nc.vector.tensor_copy(out=Et[:, t1, g * 4 : g * 4 + 4, :], in_=pt)

    for t in range(T):
        for c in range(NC_PER_T):
            ps = s_psum.tile([P, NCHUNK], f32, tag="sp")
            nc.tensor.matmul(
                ps,
                lhsT=q0t8[:, :, t * P : (t + 1) * P],
                rhs=q1t8[:, :, c * NCHUNK : (c + 1) * NCHUNK],
                start=True,
                stop=False,
                perf_mode=DR,
            )
            nc.tensor.matmul(
                ps,
                lhsT=m0r8[:, :, t * P : (t + 1) * P],
                rhs=m1r8[:, :, c * NCHUNK : (c + 1) * NCHUNK],
                start=False,
                stop=True,
                perf_mode=DR,
            )
            nc.scalar.activation(
                out=E[:, t, c * NCHUNK : (c + 1) * NCHUNK],
                in_=ps,
                func=EXP,
                scale=SCALE2,
            )
        # interleave E^T transposes of completed groups (groups 0..2); the
        # last group is emitted after out1 so PE never waits on the exp tail
        if t >= 4:
            g, t1_lo = (t - 4) // 4, ((t - 4) % 4) * 4
            emit_et_batch(g, t1_lo)

    # ---- out1 = normalized E^T @ q0 (lhsT = E tiles directly) ----
    def emit_out(lhs_slice, rhs_a, odram, mt):
        po = o_psum.tile([P, AUGW], f32, tag="op")
        for t in range(T):
            nc.tensor.matmul(
                po,
                lhsT=lhs_slice(t, mt),
                rhs=rhs_a[:, t, :],
                start=(t == 0),
                stop=(t == T - 1),
            )
        rc = small.tile([P, 1], f32, tag="rc")
        nc.vector.reciprocal(rc, po[:, D : D + 1])
        nc.vector.tensor_scalar_mul(out=rc, in0=rc, scalar1=SCALE1)
        ot = outp.tile([P, D], f32, tag="ot")
        nc.vector.tensor_scalar_mul(out=ot, in0=po[:, 0:D], scalar1=rc)
        nc.sync.dma_start(out=odram[mt * P : (mt + 1) * P, :], in_=ot)

    for mt in range(T):
        emit_out(lambda t, m: E[:, t, m * P : (m + 1) * P], q0a, out1, mt)

    # last E^T group (sources: E row tiles 12..15)
    for t1_lo in (0, 4, 8, 12):
        emit_et_batch(3, t1_lo)

    # ---- out0 = normalized E @ q1 (lhsT = Et tiles) ----
    for mt in range(T):
        emit_out(lambda t, m: Et[:, t, m, :], q1a, out0, mt)


_CACHED_NC = None


def _build():
    global _CACHED_NC
    if _CACHED_NC is not None:
        return _CACHED_NC
    nc = bacc.Bacc("TRN2", target_bir_lowering=False, debug=False)
    io = {
        "q0": nc.dram_tensor("q0", [L, D], f32, kind="ExternalInput").ap(),
        "q1": nc.dram_tensor("q1", [L, D], f32, kind="ExternalInput").ap(),
        "mask0": nc.dram_tensor("mask0", [L], i32, kind="ExternalInput").ap(),
        "mask1": nc.dram_tensor("mask1", [L], i32, kind="ExternalInput").ap(),
        "out0": nc.dram_tensor("out0", [L, D], f32, kind="ExternalOutput").ap(),
        "out1": nc.dram_tensor("out1", [L, D], f32, kind="ExternalOutput").ap(),
    }
    with tile.TileContext(nc) as tc:
        with ExitStack() as ctx:
            _emit(tc, ctx, io)
    nc.compile()
    _CACHED_NC = nc
    return nc


def run_on_cores(q0, q1, mask0, mask1, trace=False):
    """Run the SPMD kernel; returns (out0, out1, BassKernelResults)."""
    nc = _build()
    in_maps = [
        {
            "q0": np.ascontiguousarray(q0[b], dtype=np.float32),
            "q1": np.ascontiguousarray(q1[b], dtype=np.float32),
            "mask0": np.ascontiguousarray(mask0[b], dtype=np.int32),
            "mask1": np.ascontiguousarray(mask1[b], dtype=np.int32),
        }
        for b in range(B)
    ]
    br = run_bass_kernel_spmd(nc, in_maps, list(range(B)), trace=trace)
    out0 = np.stack([br.results[b]["out0"] for b in range(B)])
    out1 = np.stack([br.results[b]["out1"] for b in range(B)])
    return out0, out1, br


def kernel(q0, q1, len0=None, len1=None, mask0=None, mask1=None, **_):
    q0 = np.asarray(q0, dtype=np.float32)
    q1 = np.asarray(q1, dtype=np.float32)
    mask0 = np.asarray(mask0, dtype=np.int32)
    mask1 = np.asarray(mask1, dtype=np.int32)
    out0, out1, _br = run_on_cores(q0, q1, mask0, mask1, trace=False)
    return out0, out1


# revision 6
# speedup vs baseline: 1.2621x; 1.1935x over previous
"""Trainium2 Bass kernel for nn_Luong_61684320305412 (bidirectional masked
softmax attention, B=8, L0=L1=2048, D=256).

Sharding: data-parallel over batch B across the 8 NeuronCores (one batch
element per core). Per core:

    S   = q0 @ q1^T - 2^34 (m0 outer m1)     [fp8 DoubleRow + f32r rank-1]
    E   = exp(S / 256)                       (masked entries -> exactly 0;
                                              |S/256| <= ~0.4, no max-sub)
    out0 = (E @ q1) * (1/16) / rowsum(E)
    out1 = (E^T @ q0) * (1/16) / colsum(E)

Key facts (measured): PE streams 1 col/cycle @2.4GHz for every dtype; fp8
DoubleRow packs K=256 into one instruction (halves streamed columns for a
given contraction); per-instruction costs pipeline away when the PE queue
stays busy.

Structure:
  - Scores once in fp8 DR (q packed [d%128, d//128, l]); mask as a K=1
    f32r rank-1 matmul into the same PSUM chunk; exp on scalar -> E16 fp16.
  - E^T via regular matmuls against identity (fp32 psum); evictions fuse
    "-1" and cast to fp8 -> E8T = E^T - 1 (small values, so fp8 error is
    ~16x smaller than quantizing E directly; masked entries are exactly -1).
  - out0 (contraction over l1) in fp8 DR using E8T with the exact-mean
    identity  E @ q1 = (col-ones @ v1) + (E-1) @ q1,  v1[d] = sum_m q1[m,d]
    (v1 computed on-chip in fp16). Denominator rides in an augmented ones
    column of the fp8 q1 tiles (+2048 in v1row).
  - out1 (contraction over l0) in fp16 directly from E16 tiles.
  - Normalization: DVE reciprocal (*1/16), scalar-engine Copy with
    per-partition scale, fp32 out.
"""

import math
from contextlib import ExitStack

import numpy as np

import concourse.bass as bass
import concourse.tile as tile
from concourse import bacc, mybir
from concourse.bass_utils import run_bass_kernel_spmd
from concourse.masks import make_identity

P = 128
B = 8
L = 2048          # L0 == L1
D = 256
T = L // P        # 16 row tiles
NCHUNK = 512      # psum bank width in fp32
NC_PER_T = L // NCHUNK   # 4 chunks per row tile
AUG16 = D + 2     # 258: fp16 q tiles, ones col at D (col D+1 also ones)
AUG8 = 272        # fp8 q1 tiles padded to 16B multiple; ones col at D
MASKC = 131072.0  # 2^17: (-2^17 m0)*(2^17 m1)/256 = -2^26 -> exp == 0
SCALE2 = 1.0 / 256.0   # applied to scores inside exp
SCALE1 = 1.0 / 16.0    # applied to the averaged values at the end

f32 = mybir.dt.float32
f32r = mybir.dt.float32r
f16 = mybir.dt.float16
f8 = mybir.dt.float8e4
i32 = mybir.dt.int32
EXP = mybir.ActivationFunctionType.Exp
COPY = mybir.ActivationFunctionType.Copy
DR = mybir.MatmulPerfMode.DoubleRow


def _emit(tc: tile.TileContext, ctx: ExitStack, io: dict):
    nc = tc.nc
    q0, q1, m0, m1 = io["q0"], io["q1"], io["mask0"], io["mask1"]
    out0, out1 = io["out0"], io["out1"]

    consts = ctx.enter_context(tc.tile_pool(name="consts", bufs=1))
    qaug = ctx.enter_context(tc.tile_pool(name="qaug", bufs=1))
    qT = ctx.enter_context(tc.tile_pool(name="qT", bufs=1))
    e_pool = ctx.enter_context(tc.tile_pool(name="e", bufs=1))
    outp = ctx.enter_context(tc.tile_pool(name="outp", bufs=4))
    small = ctx.enter_context(tc.tile_pool(name="small", bufs=4))
    stage = ctx.enter_context(tc.tile_pool(name="stage", bufs=1))
    t_psum = ctx.enter_context(tc.tile_pool(name="t_psum", bufs=2, space="PSUM"))
    s_psum = ctx.enter_context(tc.tile_pool(name="s_psum", bufs=3, space="PSUM"))
    o_psum = ctx.enter_context(tc.tile_pool(name="o_psum", bufs=2, space="PSUM"))
    v_psum = ctx.enter_context(tc.tile_pool(name="v_psum", bufs=1, space="PSUM"))

    # ---- identity (fp16) for PE transposes ----
    ident_f = consts.tile([P, P], f32)
    make_identity(nc, ident_f)
    ident = consts.tile([P, P], f16)
    nc.vector.tensor_copy(out=ident, in_=ident_f)

    # ---- masks: int32 [L] -> f32r rows scaled +-2^17 (K=1 rank-1 matmul) ----
    m0i = consts.tile([1, L], i32)
    m1i = consts.tile([1, L], i32)
    nc.sync.dma_start(out=m0i, in_=m0.rearrange("(o l) -> o l", o=1))
    nc.sync.dma_start(out=m1i, in_=m1.rearrange("(o l) -> o l", o=1))
    m0r = consts.tile([1, L], f32r)
    m1r = consts.tile([1, L], f32r)
    mf = consts.tile([1, L], f32)
    nc.vector.tensor_copy(out=mf, in_=m0i)
    nc.vector.tensor_scalar_mul(out=m0r, in0=mf, scalar1=-MASKC)
    nc.vector.tensor_copy(out=mf, in_=m1i)
    nc.vector.tensor_scalar_mul(out=m1r, in0=mf, scalar1=MASKC)

    # ---- q0/q1: fp16 augmented tiles + fp8 transposed-packed tiles ----
    q0a = qaug.tile([P, T, AUG16], f16)
    q1a = qaug.tile([P, T, AUG16], f16)
    q1a8 = qaug.tile([P, T, AUG8], f8)
    q0t8 = qT.tile([P, 2, L], f8)   # [d%128, d//128, l] (DoubleRow packing)
    q1t8 = qT.tile([P, 2, L], f8)
    ones1 = consts.tile([1, P], f16)
    onescol = consts.tile([P, 1], f16)
    v1row = consts.tile([1, AUG8], f16)
    nc.vector.memset(q0a[:, :, D:AUG16], 1.0)
    nc.vector.memset(q1a[:, :, D:AUG16], 1.0)
    nc.vector.memset(q1a8[:, :, D:AUG8], 0.0)
    nc.vector.memset(q1a8[:, :, D : D + 1], 1.0)
    nc.vector.memset(ones1, 1.0)
    nc.vector.memset(onescol, 1.0)
    nc.vector.memset(v1row, 0.0)
    nc.vector.memset(v1row[:, D : D + 1], 2048.0)

    for idx, (src_dram, dst_a, dst_t8) in enumerate(
        ((q0, q0a, q0t8), (q1, q1a, q1t8))
    ):
        qf = stage.tile([P, T, D], f32, tag="qstage")
        nc.sync.dma_start(out=qf, in_=src_dram.rearrange("(t p) d -> p t d", p=P))
        nc.scalar.copy(out=dst_a[:, :, 0:D], in_=qf)
        for t in range(T):
            pt = t_psum.tile([P, 4, P], f32, tag="tp")
            for dc in range(2):
                nc.tensor.matmul(
                    pt[:, dc, :],
                    lhsT=dst_a[:, t, dc * P : (dc + 1) * P],
                    rhs=ident,
                    start=True,
                    stop=True,
                )
            nc.vector.tensor_copy(
                out=dst_t8[:, :, t * P : (t + 1) * P], in_=pt[:, 0:2, :]
            )
    # fp8 copy of q1 aug tiles (rhs of the fp8 out0 matmuls)
    nc.vector.tensor_copy(out=q1a8[:, :, 0:D], in_=q1a[:, :, 0:D])
    # v1[d] = sum_m q1[m, d] (fp16), denominator constant 2048 pre-set
    pv = v_psum.tile([1, D], f32, tag="vp")
    for t in range(T):
        nc.tensor.matmul(
            pv,
            lhsT=onescol,
            rhs=q1a[:, t, 0:D],
            start=(t == 0),
            stop=(t == T - 1),
        )
    nc.vector.tensor_copy(out=v1row[:, 0:D], in_=pv)

    # ---- S-phase (one orientation) + interleaved E^T-1 construction ----
    E16 = e_pool.tile([P, T, L], f16)        # [l0%128, l0//128, l1]
    E8T = e_pool.tile([P, T, T, P], f8)      # [l1%128, l1//128, l0//128, l0%128]

    def emit_et_batch(g, t1_lo):
        # E rows g*4..g*4+3, l1 tiles t1_lo..t1_lo+3 -> E8T = E^T - 1 (fp8)
        for t1 in range(t1_lo, t1_lo + 4):
            pt = t_psum.tile([P, 4, P], f32, tag="tp")
            for tq in range(4):
                nc.tensor.matmul(
                    pt[:, tq, :],
                    lhsT=E16[:, g * 4 + tq, t1 * P : (t1 + 1) * P],
                    rhs=ident,
                    start=True,
                    stop=True,
                )
            nc.vector.tensor_scalar_add(
                out=E8T[:, t1, g * 4 : g * 4 + 4, :], in0=pt, scalar1=-1.0
            )

    for t in range(T):
        for c in range(NC_PER_T):
            ps = s_psum.tile([P, NCHUNK], f32, tag="sp")
            nc.tensor.matmul(
                ps,
                lhsT=q0t8[:, :, t * P : (t + 1) * P],
                rhs=q1t8[:, :, c * NCHUNK : (c + 1) * NCHUNK],
                start=True,
                stop=False,
                perf_mode=DR,
            )
            nc.tensor.matmul(
                ps,
                lhsT=m0r[:, t * P : (t + 1) * P],
                rhs=m1r[:, c * NCHUNK : (c + 1) * NCHUNK],
                start=False,
                stop=True,
            )
            nc.scalar.activation(
                out=E16[:, t, c * NCHUNK : (c + 1) * NCHUNK],
                in_=ps,
                func=EXP,
                scale=SCALE2,
            )
        if t >= 4:  # groups 0..2 trail the exp wavefront
            g, t1_lo = (t - 4) // 4, ((t - 4) % 4) * 4
            emit_et_batch(g, t1_lo)

    def emit_norm(po, odram, mt):
        rc = small.tile([P, 1], f32, tag="rc")
        nc.vector.reciprocal(rc, po[:, D : D + 1])
        nc.vector.tensor_scalar_mul(out=rc, in0=rc, scalar1=SCALE1)
        ot = outp.tile([P, D], f32, tag="ot")
        nc.scalar.activation(out=ot, in_=po[:, 0:D], func=COPY, scale=rc)
        nc.sync.dma_start(out=odram[mt * P : (mt + 1) * P, :], in_=ot)

    # ---- out1 = normalized E^T @ q0: fp16, lhsT = E16 tiles directly ----
    for mt in range(T):
        po = o_psum.tile([P, AUG8], f32, tag="op")
        for t in range(T):
            nc.tensor.matmul(
                po[:, 0:AUG16],
                lhsT=E16[:, t, mt * P : (mt + 1) * P],
                rhs=q0a[:, t, :],
                start=(t == 0),
                stop=(t == T - 1),
            )
        emit_norm(po, out1, mt)

    # last E^T group (sources: E row tiles 12..15)
    for t1_lo in (0, 4, 8, 12):
        emit_et_batch(3, t1_lo)

    # ---- out0 = normalized E @ q1: fp8 DR with exact-mean correction ----
    for mt in range(T):
        po = o_psum.tile([P, AUG8], f32, tag="op")
        nc.tensor.matmul(
            po, lhsT=ones1, rhs=v1row, start=True, stop=False
        )
        for g in range(T // 2):
            nc.tensor.matmul(
                po,
                lhsT=E8T[:, 2 * g : 2 * g + 2, mt, :],
                rhs=q1a8[:, 2 * g : 2 * g + 2, :],
                start=False,
                stop=(g == T // 2 - 1),
                perf_mode=DR,
            )
        emit_norm(po, out0, mt)


_CACHED_NC = None


def _build():
    global _CACHED_NC
    if _CACHED_NC is not None:
        return _CACHED_NC
    nc = bacc.Bacc("TRN2", target_bir_lowering=False, debug=False)
    io = {
        "q0": nc.dram_tensor("q0", [L, D], f32, kind="ExternalInput").ap(),
        "q1": nc.dram_tensor("q1", [L, D], f32, kind="ExternalInput").ap(),
        "mask0": nc.dram_tensor("mask0", [L], i32, kind="ExternalInput").ap(),
        "mask1": nc.dram_tensor("mask1", [L], i32, kind="ExternalInput").ap(),
        "out0": nc.dram_tensor("out0", [L, D], f32, kind="ExternalOutput").ap(),
        "out1": nc.dram_tensor("out1", [L, D], f32, kind="ExternalOutput").ap(),
    }
    with tile.TileContext(nc) as tc:
        with ExitStack() as ctx:
            _emit(tc, ctx, io)
    nc.compile()
    _CACHED_NC = nc
    return nc


def run_on_cores(q0, q1, mask0, mask1, trace=False):
    """Run the SPMD kernel; returns (out0, out1, BassKernelResults)."""
    nc = _build()
    in_maps = [
        {
            "q0": np.ascontiguousarray(q0[b], dtype=np.float32),
            "q1": np.ascontiguousarray(q1[b], dtype=np.float32),
            "mask0": np.ascontiguousarray(mask0[b], dtype=np.int32),
            "mask1": np.ascontiguousarray(mask1[b], dtype=np.int32),
        }
        for b in range(B)
    ]
    br = run_bass_kernel_spmd(nc, in_maps, list(range(B)), trace=trace)
    out0 = np.stack([br.results[b]["out0"] for b in range(B)])
    out1 = np.stack([br.results[b]["out1"] for b in range(B)])
    return out0, out1, br


def kernel(q0, q1, len0=None, len1=None, mask0=None, mask1=None, **_):
    q0 = np.asarray(q0, dtype=np.float32)
    q1 = np.asarray(q1, dtype=np.float32)
    mask0 = np.asarray(mask0, dtype=np.int32)
    mask1 = np.asarray(mask1, dtype=np.int32)
    out0, out1, _br = run_on_cores(q0, q1, mask0, mask1, trace=False)
    return out0, out1
